# revision 6
# baseline (speedup 1.0000x reference)
"""Multi-head attention (B=2, S=2048, dim=2048, H=16, D=128) on 8 TRN2 NeuronCores.

Strategy: tensor-parallel over heads for qkv-proj + attention (each core owns
2 heads for ALL tokens, so K/V never move between cores), then one 8-core
AllToAll redistributes the per-head attention outputs to a per-token sharding,
and each core runs the output projection for its 512 tokens (no all-reduce).

Per-core bass program (SPMD, identical on all 8 cores):
  A) qkv proj: QT/KT [d, tokens] transposed + V [tokens, d] natural, bf16.
  B) attention per (head, batch): scoresT[k,q] = KT.T @ QT on PE, exp on ACT,
     denominator via DVE accumulate + GpSimd partition_all_reduce, PV on PE.
  C) AllToAll(attn_outT) -> attn_all [2048 hd, 512 tok]; out = attn_all.T @ WoutT.

Inputs are cast to bf16 on host; matmuls accumulate in fp32 PSUM; output fp32.
"""
import os
import numpy as np
import ml_dtypes

import concourse.bass as bass
import concourse.bacc as bacc
import concourse.tile as tile
import concourse.mybir as mybir
import concourse.bass_isa as bass_isa
from concourse.bass_utils import run_bass_kernel_spmd

B, S, DIM, H, D = 2, 2048, 2048, 16, 128
NC_N = 8
T = B * S                 # 4096 tokens total
TOK = T // NC_N           # 512 tokens per core (out-proj shard)
HPC = H // NC_N           # 2 heads per core
SCALE = float(D) ** -0.5

BF = mybir.dt.bfloat16
F32 = mybir.dt.float32

_CACHE: dict = {}


def _build():
    nc = bacc.Bacc("TRN2", target_bir_lowering=False, debug=False, num_devices=NC_N)
    xT_ap = nc.dram_tensor("xT", [DIM, T], BF, kind="ExternalInput").ap()
    wT_ap = nc.dram_tensor("wT", [DIM, 3 * HPC * D], BF, kind="ExternalInput").ap()
    woT_ap = nc.dram_tensor("woT", [H * D, DIM], BF, kind="ExternalInput").ap()
    out_ap = nc.dram_tensor("out", [TOK, DIM], F32, kind="ExternalOutput").ap()

    P = 128
    DC = DIM // P            # 16 contraction chunks
    NTC = T // 512           # 8 token chunks of 512
    QCOL = HPC * D           # 256 q/k/v columns per core

    with tile.TileContext(nc) as tc:
        with tc.tile_pool(name="persist", bufs=1) as persist, \
             tc.tile_pool(name="dram", bufs=1, space="DRAM") as dram:

            # persistent SBUF tensors
            qt_sb = persist.tile([P, HPC, T], BF, tag="qt")      # Q^T
            kt_sb = persist.tile([P, HPC, T], BF, tag="kt")      # K^T
            v_sb = persist.tile([P, T // P, QCOL], BF, tag="v")  # V natural [tok, hd]
            attn_sb = persist.tile([P, HPC, T], BF, tag="attn")  # normalized attn^T
            wo_sb = persist.tile([P, H * D // P, DIM], BF, tag="wo")

            # A2A bounce buffers
            a2a_in = dram.tile([NC_N * HPC * D, TOK], BF, tag="a2ain")
            a2a_out = dram.tile([H * D, TOK], BF, tag="a2aout")

            # ---- Stage A: qkv projection ----
            with tc.tile_pool(name="w", bufs=1) as wpool, \
                 tc.tile_pool(name="xin", bufs=2) as xpool, \
                 tc.tile_pool(name="psqk", bufs=4, space="PSUM") as psqk, \
                 tc.tile_pool(name="psv", bufs=4, space="PSUM") as psv:
                w_sb = wpool.tile([P, DC, 3 * QCOL], BF)
                nc.sync.dma_start(
                    out=w_sb[:], in_=wT_ap.rearrange("(dc p) c -> p dc c", p=P))

                for t in range(NTC):
                    xt = xpool.tile([P, DC, 512], BF, tag="xt")
                    nc.sync.dma_start(
                        out=xt[:],
                        in_=xT_ap.rearrange("(dc p) n -> p dc n", p=P)[
                            :, :, t * 512:(t + 1) * 512])
                    # Q^T and K^T (transposed): lhsT = w tile, rhs = x tile
                    for oc in range(2 * HPC):
                        ps = psqk.tile([P, 512], F32, tag="psqk")
                        for dc in range(DC):
                            nc.tensor.matmul(
                                ps[:],
                                w_sb[:, dc, oc * P:(oc + 1) * P],
                                xt[:, dc, :],
                                start=(dc == 0), stop=(dc == DC - 1))
                        dst = qt_sb if oc < HPC else kt_sb
                        hc = oc % HPC
                        nc.scalar.activation(
                            dst[:, hc, t * 512:(t + 1) * 512], ps[:],
                            mybir.ActivationFunctionType.Copy)
                    # V (natural): lhsT = x sub-tile, rhs = w
                    for ts in range(4):
                        psb = psv.tile([P, QCOL], F32, tag="psv")
                        for dc in range(DC):
                            nc.tensor.matmul(
                                psb[:],
                                xt[:, dc, ts * P:(ts + 1) * P],
                                w_sb[:, dc, 2 * QCOL:3 * QCOL],
                                start=(dc == 0), stop=(dc == DC - 1))
                        nc.scalar.activation(
                            v_sb[:, t * 4 + ts, :], psb[:],
                            mybir.ActivationFunctionType.Copy)

            # preload Wout^T during attention
            nc.sync.dma_start(
                out=wo_sb[:], in_=woT_ap.rearrange("(hc p) d -> p hc d", p=P))

            # ---- Stage B: attention per (head, batch) ----
            with tc.tile_pool(name="exp", bufs=4) as epool, \
                 tc.tile_pool(name="accp", bufs=2) as apool, \
                 tc.tile_pool(name="pss", bufs=2, space="PSUM") as pss, \
                 tc.tile_pool(name="psa", bufs=2, space="PSUM") as psa:
                KC = S // P   # 16 key chunks
                for h in range(HPC):
                    for b in range(B):
                        t0 = b * S
                        for qh in range(2):       # q halves of 1024
                            q0 = t0 + qh * 1024
                            ps_attn = psa.tile([P, 1024], F32, tag="psa")
                            acc = apool.tile([P, 1024], F32, tag="acc")
                            for kc in range(KC):
                                ps_s = pss.tile([P, 1024], F32, tag="pss")
                                kslice = kt_sb[:, h, t0 + kc * P: t0 + (kc + 1) * P]
                                for qs in range(2):
                                    nc.tensor.matmul(
                                        ps_s[:, qs * 512:(qs + 1) * 512],
                                        kslice,
                                        qt_sb[:, h, q0 + qs * 512: q0 + (qs + 1) * 512],
                                        start=True, stop=True)
                                et = epool.tile([P, 1024], BF, tag="exp")
                                nc.scalar.activation(
                                    et[:], ps_s[:],
                                    mybir.ActivationFunctionType.Exp, scale=SCALE)
                                if kc == 0:
                                    nc.vector.tensor_copy(out=acc[:], in_=et[:])
                                else:
                                    nc.vector.tensor_tensor(
                                        out=acc[:], in0=acc[:], in1=et[:],
                                        op=mybir.AluOpType.add)
                                vslice = v_sb[:, b * (S // P) + kc, h * D:(h + 1) * D]
                                for qs in range(2):
                                    nc.tensor.matmul(
                                        ps_attn[:, qs * 512:(qs + 1) * 512],
                                        vslice,
                                        et[:, qs * 512:(qs + 1) * 512],
                                        start=(kc == 0), stop=(kc == KC - 1))
                            sums = apool.tile([P, 1024], F32, tag="sums")
                            nc.gpsimd.partition_all_reduce(
                                sums[:], acc[:], channels=P,
                                reduce_op=bass_isa.ReduceOp.add)
                            rb = apool.tile([P, 1024], F32, tag="rb")
                            nc.vector.reciprocal_approx_fast(out=rb[:], in_=sums[:])
                            nc.vector.tensor_tensor(
                                out=attn_sb[:, h, q0:q0 + 1024],
                                in0=ps_attn[:], in1=rb[:],
                                op=mybir.AluOpType.mult)

            # ---- Stage C: AllToAll + output projection ----
            for j in range(NC_N):
                nc.sync.dma_start(
                    out=a2a_in[j * HPC * D:(j + 1) * HPC * D, :].rearrange(
                        "(hc p) f -> p hc f", p=P),
                    in_=attn_sb[:, :, j * TOK:(j + 1) * TOK])
            nc.gpsimd.collective_compute(
                "AllToAll", mybir.AluOpType.bypass,
                replica_groups=[list(range(NC_N))],
                ins=[a2a_in.opt()], outs=[a2a_out.opt()])

            with tc.tile_pool(name="attall", bufs=1) as allpool, \
                 tc.tile_pool(name="outp", bufs=3) as outpool, \
                 tc.tile_pool(name="psc", bufs=4, space="PSUM") as psc:
                attn_all = allpool.tile([P, H * D // P, TOK], BF)
                nc.sync.dma_start(
                    out=attn_all[:],
                    in_=a2a_out.rearrange("(hc p) f -> p hc f", p=P))
                out_view = out_ap.rearrange("(qs p) d -> p qs d", p=P)
                HC = H * D // P  # 16
                for qs in range(TOK // P):       # 4
                    for ds in range(DIM // 512):  # 4
                        ps = psc.tile([P, 512], F32, tag="psc")
                        for hc in range(HC):
                            nc.tensor.matmul(
                                ps[:],
                                attn_all[:, hc, qs * P:(qs + 1) * P],
                                wo_sb[:, hc, ds * 512:(ds + 1) * 512],
                                start=(hc == 0), stop=(hc == HC - 1))
                        ot = outpool.tile([P, 512], F32, tag="ot")
                        nc.scalar.activation(
                            ot[:], ps[:], mybir.ActivationFunctionType.Copy)
                        nc.sync.dma_start(
                            out=out_view[:, qs, ds * 512:(ds + 1) * 512],
                            in_=ot[:])

    nc.compile()
    return nc


def _get_nc():
    if "nc" not in _CACHE:
        if os.environ.get("KERNEL_TRACE"):
            try:
                import axon_profile_shim
                axon_profile_shim.install()
            except Exception:
                pass
        _CACHE["nc"] = _build()
    return _CACHE["nc"]


def kernel(x, Wqkv, Wout):
    nc = _get_nc()

    xb = np.asarray(x, np.float32).reshape(T, DIM)
    xT = np.ascontiguousarray(xb.T).astype(ml_dtypes.bfloat16)
    Wqkv = np.asarray(Wqkv, np.float32)
    woT = np.ascontiguousarray(np.asarray(Wout, np.float32).T).astype(
        ml_dtypes.bfloat16)

    in_maps = []
    for c in range(NC_N):
        wq = Wqkv[HPC * D * c: HPC * D * (c + 1)]
        wk = Wqkv[H * D + HPC * D * c: H * D + HPC * D * (c + 1)]
        wv = Wqkv[2 * H * D + HPC * D * c: 2 * H * D + HPC * D * (c + 1)]
        wT = np.ascontiguousarray(
            np.concatenate([wq, wk, wv], axis=0).T).astype(ml_dtypes.bfloat16)
        in_maps.append({"xT": xT, "wT": wT, "woT": woT})

    trace = bool(os.environ.get("KERNEL_TRACE"))
    res = run_bass_kernel_spmd(
        nc, in_maps, core_ids=list(range(NC_N)), trace=trace)
    _CACHE["exec_time_ns"] = res.exec_time_ns

    out = np.concatenate(
        [res.results[c]["out"] for c in range(NC_N)], axis=0)
    return out.reshape(B, S, DIM).astype(np.float32)


# revision 12
# speedup vs baseline: 1.0692x; 1.0692x over previous
"""Multi-head attention (B=2, S=2048, dim=2048, H=16, D=128) on 8 TRN2 NeuronCores.

Strategy: tensor-parallel over heads for qkv-proj + attention (each core owns
2 heads for ALL tokens, so K/V never move between cores), then 8-core
AllToAlls (one per local head, overlapped with attention) redistribute the
per-head attention outputs to a per-token sharding, and each core runs the
output projection for its 512 tokens (no all-reduce).

Per-core bass program (SPMD, identical on all 8 cores):
  A) qkv proj: QT/KT [d, tokens] transposed + V [tokens, d] natural, bf16.
  B) attention per (head, batch): scoresT[k,q] = KT.T @ QT on PE, exp on ACT,
     PV on PE; raw attn evicted to SBUF, then normalized by 1/rowsum
     (DVE accumulate + GpSimd partition_all_reduce) off the critical path.
  C) AllToAll per head -> attn_all [hd, 512 tok]; out = attn_all.T @ WoutT.

Inputs are cast to bf16 on host; matmuls accumulate in fp32 PSUM; output fp32.
"""
import os
import numpy as np
import ml_dtypes

import concourse.bass as bass
import concourse.bacc as bacc
import concourse.tile as tile
import concourse.mybir as mybir
import concourse.bass_isa as bass_isa
from concourse.bass_utils import run_bass_kernel_spmd

B, S, DIM, H, D = 2, 2048, 2048, 16, 128
NC_N = 8
T = B * S                 # 4096 tokens total
TOK = T // NC_N           # 512 tokens per core (out-proj shard)
HPC = H // NC_N           # 2 heads per core
SCALE = float(D) ** -0.5

BF = mybir.dt.bfloat16
F32 = mybir.dt.float32

_CACHE: dict = {}


def _build():
    nc = bacc.Bacc("TRN2", target_bir_lowering=False, debug=False, num_devices=NC_N)
    xT_ap = nc.dram_tensor("xT", [DIM, T], BF, kind="ExternalInput").ap()
    wT_ap = nc.dram_tensor("wT", [DIM, 3 * HPC * D], BF, kind="ExternalInput").ap()
    woT_ap = nc.dram_tensor("woT", [H * D, DIM], BF, kind="ExternalInput").ap()
    out_ap = nc.dram_tensor("out", [TOK, DIM], F32, kind="ExternalOutput").ap()

    P = 128
    DC = DIM // P            # 16 contraction chunks
    QCOL = HPC * D           # 256 q/k/v columns per core

    with tile.TileContext(nc) as tc:
        with tc.tile_pool(name="persist", bufs=1) as persist, \
             tc.tile_pool(name="dram", bufs=1, space="DRAM") as dram:

            # persistent SBUF tensors
            qt_sb = persist.tile([P, HPC, T], BF, tag="qt")      # Q^T
            kt_sb = persist.tile([P, HPC, T], BF, tag="kt")      # K^T
            v_sb = persist.tile([P, T // P, QCOL], BF, tag="v")  # V natural [tok, hd]
            attn_sb = persist.tile([P, HPC, T], BF, tag="attn")  # normalized attn^T

            # A2A bounce buffers, one pair per local head
            a2a_in = [dram.tile([NC_N * D, TOK], BF, tag=f"a2ain{h}", name=f"a2ain{h}")
                      for h in range(HPC)]
            a2a_out = [dram.tile([NC_N * D, TOK], BF, tag=f"a2aout{h}", name=f"a2aout{h}")
                       for h in range(HPC)]

            # ---- Stage A: qkv projection ----
            with tc.tile_pool(name="w", bufs=1) as wpool, \
                 tc.tile_pool(name="xin", bufs=4) as xpool, \
                 tc.tile_pool(name="psA", bufs=4, space="PSUM") as psA:
                w_sb = wpool.tile([P, DC, 3 * QCOL], BF)
                nc.sync.dma_start(
                    out=w_sb[:], in_=wT_ap.rearrange("(dc p) c -> p dc c", p=P))

                for t in range(T // 1024):       # 4 token chunks of 1024
                    xth = []
                    for half in range(2):
                        xh = xpool.tile([P, DC, 512], BF, tag="xt",
                                        name=f"xt{t}_{half}")
                        nc.sync.dma_start(
                            out=xh[:],
                            in_=xT_ap.rearrange("(dc p) n -> p dc n", p=P)[
                                :, :, t * 1024 + half * 512:
                                t * 1024 + (half + 1) * 512])
                        xth.append(xh)
                    # Q^T and K^T (transposed): one stationary, two 512 moves
                    for oc in range(2 * HPC):
                        ps = psA.tile([P, 1024], F32, tag="ps")
                        for dc in range(DC):
                            for half in range(2):
                                nc.tensor.matmul(
                                    ps[:, half * 512:(half + 1) * 512],
                                    w_sb[:, dc, oc * P:(oc + 1) * P],
                                    xth[half][:, dc, :],
                                    start=(dc == 0), stop=(dc == DC - 1))
                        dst = qt_sb if oc < HPC else kt_sb
                        hc = oc % HPC
                        nc.scalar.activation(
                            dst[:, hc, t * 1024:(t + 1) * 1024], ps[:],
                            mybir.ActivationFunctionType.Copy)
                    # V (natural): lhsT = x sub-tile, rhs = w
                    for vg in range(2):          # two psum tiles of 4 t-subs
                        psb = psA.tile([P, 1024], F32, tag="ps")
                        for ts in range(4):
                            tsub = vg * 4 + ts
                            for dc in range(DC):
                                nc.tensor.matmul(
                                    psb[:, ts * QCOL:(ts + 1) * QCOL],
                                    xth[tsub // 4][:, dc, (tsub % 4) * P:
                                                   (tsub % 4 + 1) * P],
                                    w_sb[:, dc, 2 * QCOL:3 * QCOL],
                                    start=(dc == 0), stop=(dc == DC - 1))
                        for ts in range(4):
                            tsub = vg * 4 + ts
                            nc.scalar.activation(
                                v_sb[:, t * 8 + tsub, :],
                                psb[:, ts * QCOL:(ts + 1) * QCOL],
                                mybir.ActivationFunctionType.Copy)

            # Wout^T, loaded during attention (own pool so its SBUF space
            # is disjoint from stage A's w/x pools)
            wop_cm = tc.tile_pool(name="wop", bufs=1)
            wopool = wop_cm.__enter__()
            wo_sb = wopool.tile([P, H * D // P, DIM], BF, tag="wo")
            nc.sync.dma_start(
                out=wo_sb[:], in_=woT_ap.rearrange("(hc p) d -> p hc d", p=P))

            # ---- Stage B: attention per (head, batch) + per-head A2A ----
            with tc.tile_pool(name="exp", bufs=4) as epool, \
                 tc.tile_pool(name="accp", bufs=2) as apool, \
                 tc.tile_pool(name="raw", bufs=3) as rawpool, \
                 tc.tile_pool(name="pss", bufs=2, space="PSUM") as pss, \
                 tc.tile_pool(name="psa", bufs=2, space="PSUM") as psa:
                KC = S // P   # 16 key chunks
                for h in range(HPC):
                    for b in range(B):
                        t0 = b * S
                        for qh in range(2):       # q halves of 1024
                            q0 = t0 + qh * 1024
                            ps_attn = psa.tile([P, 1024], F32, tag="psa")
                            acc = apool.tile([P, 1024], F32, tag="acc")
                            for kc in range(KC):
                                ps_s = pss.tile([P, 1024], F32, tag="pss")
                                kslice = kt_sb[:, h, t0 + kc * P: t0 + (kc + 1) * P]
                                for qs in range(2):
                                    nc.tensor.matmul(
                                        ps_s[:, qs * 512:(qs + 1) * 512],
                                        kslice,
                                        qt_sb[:, h, q0 + qs * 512: q0 + (qs + 1) * 512],
                                        start=True, stop=True)
                                et = epool.tile([P, 1024], BF, tag="exp")
                                nc.scalar.activation(
                                    et[:], ps_s[:],
                                    mybir.ActivationFunctionType.Exp, scale=SCALE)
                                if kc == 0:
                                    nc.vector.tensor_copy(out=acc[:], in_=et[:])
                                else:
                                    nc.vector.tensor_tensor(
                                        out=acc[:], in0=acc[:], in1=et[:],
                                        op=mybir.AluOpType.add)
                                vslice = v_sb[:, b * (S // P) + kc, h * D:(h + 1) * D]
                                for qs in range(2):
                                    nc.tensor.matmul(
                                        ps_attn[:, qs * 512:(qs + 1) * 512],
                                        vslice,
                                        et[:, qs * 512:(qs + 1) * 512],
                                        start=(kc == 0), stop=(kc == KC - 1))
                            # evict raw attn so PSUM frees without waiting on
                            # the normalization chain
                            araw = rawpool.tile([P, 1024], F32, tag="araw")
                            nc.scalar.activation(
                                araw[:], ps_attn[:],
                                mybir.ActivationFunctionType.Copy)
                            sums = apool.tile([P, 1024], F32, tag="sums")
                            nc.gpsimd.partition_all_reduce(
                                sums[:], acc[:], channels=P,
                                reduce_op=bass_isa.ReduceOp.add)
                            rb = apool.tile([P, 1024], F32, tag="rb")
                            nc.vector.reciprocal_approx_fast(out=rb[:], in_=sums[:])
                            nc.vector.tensor_tensor(
                                out=attn_sb[:, h, q0:q0 + 1024],
                                in0=araw[:], in1=rb[:],
                                op=mybir.AluOpType.mult)
                    # this head fully done on all cores at the same program
                    # point -> fire its AllToAll while the next head computes
                    for j in range(NC_N):
                        nc.sync.dma_start(
                            out=a2a_in[h][j * D:(j + 1) * D, :].rearrange(
                                "(one p) f -> p one f", p=P),
                            in_=attn_sb[:, h:h + 1, j * TOK:(j + 1) * TOK])
                    nc.gpsimd.collective_compute(
                        "AllToAll", mybir.AluOpType.bypass,
                        replica_groups=[list(range(NC_N))],
                        ins=[a2a_in[h].opt()], outs=[a2a_out[h].opt()])

            # ---- Stage C: output projection ----
            with tc.tile_pool(name="attall", bufs=1) as allpool, \
                 tc.tile_pool(name="outp", bufs=3) as outpool, \
                 tc.tile_pool(name="psc", bufs=4, space="PSUM") as psc:
                # attn_all[h] rows i*128+p = global head (2i+h), dim p
                attn_all = [allpool.tile([P, NC_N, TOK], BF, tag=f"al{h}", name=f"al{h}")
                            for h in range(HPC)]
                for h in range(HPC):
                    nc.sync.dma_start(
                        out=attn_all[h][:],
                        in_=a2a_out[h].rearrange("(i p) f -> p i f", p=P))
                out_view = out_ap.rearrange("(qs p) d -> p qs d", p=P)
                for qs in range(TOK // P):       # 4
                    pss_c = [psc.tile([P, 512], F32, tag="psc", name=f"psc{qs}_{d_}") for d_ in range(4)]
                    for h in range(HPC):         # accumulate h=0 heads first
                        for i in range(NC_N):
                            g = 2 * i + h        # global head = wo row chunk
                            first = (h == 0 and i == 0)
                            last = (h == HPC - 1 and i == NC_N - 1)
                            for ds in range(4):
                                nc.tensor.matmul(
                                    pss_c[ds][:],
                                    attn_all[h][:, i, qs * P:(qs + 1) * P],
                                    wo_sb[:, g, ds * 512:(ds + 1) * 512],
                                    start=first, stop=last)
                    for ds in range(4):
                        ot = outpool.tile([P, 512], F32, tag="ot")
                        nc.scalar.activation(
                            ot[:], pss_c[ds][:], mybir.ActivationFunctionType.Copy)
                        nc.sync.dma_start(
                            out=out_view[:, qs, ds * 512:(ds + 1) * 512],
                            in_=ot[:])
            wop_cm.__exit__(None, None, None)

    nc.compile()
    return nc


def _get_nc():
    if "nc" not in _CACHE:
        if os.environ.get("KERNEL_TRACE"):
            try:
                import axon_profile_shim
                axon_profile_shim.install()
            except Exception:
                pass
        _CACHE["nc"] = _build()
    return _CACHE["nc"]


def kernel(x, Wqkv, Wout):
    nc = _get_nc()

    xb = np.asarray(x, np.float32).reshape(T, DIM)
    xT = np.ascontiguousarray(xb.T).astype(ml_dtypes.bfloat16)
    Wqkv = np.asarray(Wqkv, np.float32)
    woT = np.ascontiguousarray(np.asarray(Wout, np.float32).T).astype(
        ml_dtypes.bfloat16)

    in_maps = []
    for c in range(NC_N):
        wq = Wqkv[HPC * D * c: HPC * D * (c + 1)]
        wk = Wqkv[H * D + HPC * D * c: H * D + HPC * D * (c + 1)]
        wv = Wqkv[2 * H * D + HPC * D * c: 2 * H * D + HPC * D * (c + 1)]
        wT = np.ascontiguousarray(
            np.concatenate([wq, wk, wv], axis=0).T).astype(ml_dtypes.bfloat16)
        in_maps.append({"xT": xT, "wT": wT, "woT": woT})

    trace = bool(os.environ.get("KERNEL_TRACE"))
    res = run_bass_kernel_spmd(
        nc, in_maps, core_ids=list(range(NC_N)), trace=trace)
    _CACHE["exec_time_ns"] = res.exec_time_ns

    out = np.concatenate(
        [res.results[c]["out"] for c in range(NC_N)], axis=0)
    return out.reshape(B, S, DIM).astype(np.float32)
